# revision 43
# baseline (speedup 1.0000x reference)
"""Trainium2 Bass kernel for nn_Agent_40063454937396 (LSTM agent decode+sample).

Strategy (8 NeuronCores, model-parallel):
  - Hidden dim H=1024 sharded 8-way: each core computes gates/cell for its 128
    hidden dims over the whole batch, then AllGathers the new hidden state.
  - Vocab V=8192 sharded 8-way for the output projection: each core computes
    logits for its 1024-vocab shard, computes local softmax partials and a
    local Gumbel argmax candidate, then AllGathers 6 scalars per row to
    reproduce exact global sampling + log-prob + entropy.
  - Batch 512 split into 4 independent quarters of 128 rows; the four
    per-quarter chains are pipelined so PE matmuls of one quarter hide the
    collective/sampling latency of the others.
  - All matmuls are dual-bf16 (hi/lo split, 3 terms) for fp32-level accuracy
    at 3 cycles/row instead of fp32's 4.
  - Gumbel noise is precomputed on host CPU with the exact JAX PRNG the
    reference uses (jax.random.gumbel(key, (B, V)) per step).
"""

import os
import sys

for _p in ("/opt/trn_rl_repo", "/root/.axon_site/_ro/trn_rl_repo", "/root/.axon_site"):
    if os.path.isdir(_p) and _p not in sys.path:
        sys.path.append(_p)

# Need the CPU backend alongside axon for host-side PRNG reproduction.
_plat = os.environ.get("JAX_PLATFORMS", "")
if _plat and "cpu" not in _plat:
    os.environ["JAX_PLATFORMS"] = _plat + ",cpu"

import numpy as np
import ml_dtypes

import concourse.bass as bass
import concourse.mybir as mybir
import concourse.tile as tile
from concourse.bass import IndirectOffsetOnAxis
from concourse.bass_utils import run_bass_kernel_spmd
from concourse.vector_clock import ScopedClock

# ---------------------------------------------------------------------------
# The agent image lacks antenv.axon_hooks; bass_utils imports it when
# trace=True under axon. Provide a functional stand-in (NTFF profiling via
# ctypes into libaxon_pjrt.so).
def _install_axon_hooks_shim():
    import contextlib
    import ctypes
    import types

    if "antenv.axon_hooks" in sys.modules:
        return
    try:
        import antenv  # noqa: F401
    except ImportError:
        return
    holder = {"hook": None}

    def set_hook(h):
        holder["hook"] = h

    def get_hook():
        return holder["hook"]

    mod = types.ModuleType("antenv.axon_hooks")
    mod.set_axon_ntff_profile_hook = set_hook
    mod.get_axon_ntff_profile_hook = get_hook
    sys.modules["antenv.axon_hooks"] = mod
    sys.modules["antenv"].axon_hooks = mod

    so_path = "/opt/axon/libaxon_pjrt.so"
    if not os.path.exists(so_path):
        return
    try:
        lib = ctypes.CDLL(so_path)
    except OSError:
        return
    if not hasattr(lib, "axon_start_nrt_profile"):
        return
    lib.axon_start_nrt_profile.argtypes = [
        ctypes.POINTER(ctypes.c_int64), ctypes.c_size_t]
    lib.axon_start_nrt_profile.restype = ctypes.c_int64
    lib.axon_stop_nrt_profile.argtypes = [ctypes.c_char_p]
    lib.axon_stop_nrt_profile.restype = ctypes.c_int64

    @contextlib.contextmanager
    def _hook(output_dir, device_ids):
        import jax
        jax.devices()
        if device_ids:
            ids = (ctypes.c_int64 * len(device_ids))(*device_ids)
            rc = lib.axon_start_nrt_profile(ids, len(device_ids))
        else:
            rc = lib.axon_start_nrt_profile(None, 0)
        if rc != 0:
            raise RuntimeError(f"axon_start_nrt_profile rc={rc}")
        try:
            yield
        finally:
            n = lib.axon_stop_nrt_profile(str(output_dir).encode())
            print(f"profile: {n} file(s) written to {output_dir}", file=sys.stderr)

    set_hook(_hook)


_install_axon_hooks_shim()

F32 = mybir.dt.float32
BF16 = mybir.dt.bfloat16
I32 = mybir.dt.int32
AF = mybir.ActivationFunctionType
OP = mybir.AluOpType

NCORES = 8
B, F_IN, H, E, V = 512, 576, 1024, 256, 8192
HS = H // NCORES          # 128 hidden dims per core
VS = V // NCORES          # 1024 vocab per core
NQ = 4                    # batch quarters
QB = B // NQ              # 128 rows per quarter
KH = H // 128             # 8 contraction chunks over H
KE = E // 128             # 2 contraction chunks over E
KF = 5                    # padded 576 -> 640 = 5 chunks over F_IN
TERMS = ((0, 0), (0, 1), (1, 0))  # (lhs hi/lo, rhs hi/lo) dual-bf16 product terms
RG = [list(range(NCORES))]

bf16 = ml_dtypes.bfloat16


# ---------------------------------------------------------------------------
# walrus rejects instructions with >1 sync wait on the kernel-tail drain;
# split the auto-generated drain waits across multiple drain instructions.
def _patched_drain_and_barrier(self, tick_clock, wait_clock):
    nc = self.nc
    drain_inst = nc.sync.drain()
    wait_clock.add_sem_waits(
        drain_inst.ins, ScopedClock({None: tick_clock.global_clock})
    )
    si = drain_inst.ins.sync_info
    waits = list(si.on_wait)
    if len(waits) > 1:
        si.on_wait = waits[:1]
        for w in waits[1:]:
            d2 = nc.sync.drain()
            d2.ins.sync_info = mybir.SyncInfo(on_wait=[w], on_update=[])
    nc.all_engine_barrier()
    assert self.sems is not None
    popped = nc._tile_sem_poison_stack.pop()
    assert popped is self._sem_poison
    sems = list(self.sems.allocated().values())
    # EVENT_SEMAPHORE_RANGE_CLEAR rejects ranges wider than 16
    for i in range(0, len(sems), 8):
        nc.clear_and_free_semaphores(sems[i:i + 8])
    nc.all_engine_barrier()


tile.TileContext._drain_and_barrier = _patched_drain_and_barrier


def _split_excess_waits(nc, max_waits=1):
    """walrus rejects instructions carrying more than one sync wait; move
    excess waits onto standalone EventSemaphore carriers inserted before."""
    for bb in nc.m.functions[0].blocks:
        il = bb.instructions
        out = []
        changed = False
        for ins in il:
            si = ins.sync_info
            waits = list(si.on_wait) if si else []
            if len(waits) > max_waits:
                changed = True
                keep = waits[:max_waits]
                rest = waits[max_waits:]
                for j, w in enumerate(rest):
                    carrier = mybir.InstNoOp(
                        name=f"{ins.name}_xw{j}", ins=[], outs=[],
                        bass_nofuse=True)
                    carrier.engine = ins.engine
                    carrier.sync_info = mybir.SyncInfo(on_wait=[w], on_update=[])
                    out.append(carrier)
                ins.sync_info = mybir.SyncInfo(
                    on_wait=keep, on_update=list(si.on_update))
            out.append(ins)
        if changed:
            bb.instructions = out


# ---------------------------------------------------------------------------
def split_pair(x):
    """fp32 array -> (hi, lo) bf16 arrays with hi+lo ~= x (2^-17 accurate)."""
    x = np.asarray(x, np.float32)
    hi = x.astype(bf16)
    lo = (x - hi.astype(np.float32)).astype(bf16)
    return hi, lo


SPLIT_WAITS = True  # set False when running under CoreSim (it rejects carriers)


def build_program(steps: int, has_gbias: bool = True, has_bout: bool = True,
                  has_fcb: bool = True):
    nc = bass.Bass(target_bir_lowering=False, trn_type="TRN2")

    dt = nc.dram_tensor
    whh_hi = dt("whh_hi", [128, KH, 512], BF16, kind="ExternalInput")
    whh_lo = dt("whh_lo", [128, KH, 512], BF16, kind="ExternalInput")
    wih_hi = dt("wih_hi", [128, KE, 512], BF16, kind="ExternalInput")
    wih_lo = dt("wih_lo", [128, KE, 512], BF16, kind="ExternalInput")
    gbias_d = dt("gbias", [1, 2, 512], BF16, kind="ExternalInput")
    wout_hi = dt("wout_hi", [128, KH, VS], BF16, kind="ExternalInput")
    wout_lo = dt("wout_lo", [128, KH, VS], BF16, kind="ExternalInput")
    bout_d = dt("bout", [1, 2, VS], BF16, kind="ExternalInput")
    fcw_hi = dt("fcw_hi", [128, KF, KH, 128], BF16, kind="ExternalInput")
    fcw_lo = dt("fcw_lo", [128, KF, KH, 128], BF16, kind="ExternalInput")
    fcb_d = dt("fcb", [1, KH, 2, 128], BF16, kind="ExternalInput")
    fv_hi = dt("fv_hi", [128, KF, B], BF16, kind="ExternalInput")
    fv_lo = dt("fv_lo", [128, KF, B], BF16, kind="ExternalInput")
    emb_d = dt("embp", [V, 2 * E], BF16, kind="ExternalInput")
    et0_d = dt("et0", [128, KE, 2, QB], BF16, kind="ExternalInput")
    gum_d = dt("gumb", [steps, 128, NQ, VS], F32, kind="ExternalInput")
    iota_d = dt("iotav", [128, VS], F32, kind="ExternalInput")
    crev_d = dt("crev", [128, NCORES], F32, kind="ExternalInput")
    ones_d = dt("onesb", [1, 512], BF16, kind="ExternalInput")
    idbf_d = dt("idbf", [128, 128], BF16, kind="ExternalInput")
    idf32_d = dt("idf32", [128, 128], F32, kind="ExternalInput")

    msg_d = dt("msg", [B, steps + 1], I32, kind="ExternalOutput")
    lp_d = dt("lp", [B, steps + 1], F32, kind="ExternalOutput")
    ent_d = dt("ent", [B, steps + 1], F32, kind="ExternalOutput")

    mm = nc.tensor.matmul

    with tile.TileContext(nc) as tc:
        with (
            tc.tile_pool(name="wp", bufs=1) as wp,
            tc.tile_pool(name="st", bufs=1) as st,
            tc.tile_pool(name="gp", bufs=2) as gp,
            tc.tile_pool(name="wk", bufs=3) as wk,
            tc.tile_pool(name="sm", bufs=4) as sm,
            tc.tile_pool(name="pgp", bufs=4, space="PSUM") as pgp,
            tc.tile_pool(name="plp", bufs=2, space="PSUM") as plp,
            tc.tile_pool(name="ptp", bufs=2, space="PSUM") as ptp,
            tc.tile_pool(name="dr", bufs=2, space="DRAM") as dr,
        ):
            # ---- persistent weight tiles ----
            whh = [wp.tile([128, KH, 512], BF16, name=f"whh{i}") for i in range(2)]
            wih = [wp.tile([128, KE, 512], BF16, name=f"wih{i}") for i in range(2)]
            wout = [wp.tile([128, KH, VS], BF16, name=f"wout{i}") for i in range(2)]
            gbias = wp.tile([1, 2, 512], BF16, name="gbias")
            bout = wp.tile([1, 2, VS], BF16, name="bout")
            iota = wp.tile([128, VS], F32, name="iota")
            crev = wp.tile([128, NCORES], F32, name="crev")
            onesb = wp.tile([1, 512], BF16, name="onesb")
            idbf = wp.tile([128, 128], BF16, name="idbf")
            idf32 = wp.tile([128, 128], F32, name="idf32")

            for t_, d_ in (
                (whh[0], whh_hi), (whh[1], whh_lo),
                (wih[0], wih_hi), (wih[1], wih_lo),
                (wout[0], wout_hi), (wout[1], wout_lo),
                (gbias, gbias_d), (bout, bout_d), (iota, iota_d),
                (crev, crev_d), (onesb, ones_d), (idbf, idbf_d),
                (idf32, idf32_d),
            ):
                nc.sync.dma_start(t_[:], d_[:])

            # ---- persistent state ----
            htq = [st.tile([128, KH, 2, QB], BF16, name=f"htq{q}") for q in range(NQ)]
            etq = [st.tile([128, KE, 2, QB], BF16, name=f"etq{q}") for q in range(NQ)]
            ctq = [st.tile([128, QB], F32, name=f"ctq{q}") for q in range(NQ)]
            tokbuf = st.tile([128, NQ, steps + 1], I32, name="tokbuf")
            lpbuf = st.tile([128, NQ, steps + 1], F32, name="lpbuf")
            entbuf = st.tile([128, NQ, steps + 1], F32, name="entbuf")

            for q in range(NQ):
                nc.vector.memset(ctq[q][:], 0.0)
                nc.sync.dma_start(etq[q][:], et0_d[:])
            nc.vector.memset(tokbuf[:], 0)
            nc.vector.memset(lpbuf[:], 0.0)
            nc.vector.memset(entbuf[:], 0.0)

            # ---- init: ht0 = fc(feature_vector) (h-major, full batch) ----
            with tc.tile_pool(name="ip", bufs=1) as ip:
                fcb = ip.tile([1, KH, 2, 128], BF16, name="fcb")
                fv = [ip.tile([128, KF, B], BF16, name=f"fv{i}") for i in range(2)]
                for t_, d_ in ((fcb, fcb_d), (fv[0], fv_hi), (fv[1], fv_lo)):
                    nc.sync.dma_start(t_[:], d_[:])

                for hc in range(KH):
                    fcw = [ip.tile([128, KF, 128], BF16, tag=f"fcw{i}", bufs=2,
                                   name=f"fcw{i}_{hc}") for i in range(2)]
                    nc.sync.dma_start(fcw[0][:], fcw_hi[:, :, hc, :])
                    nc.sync.dma_start(fcw[1][:], fcw_lo[:, :, hc, :])
                    ph = pgp.tile([128, B], F32, tag="pg", name=f"ph{hc}")
                    ops = [(fcw[a][:, k, :], fv[b2][:, k, :])
                           for (a, b2) in TERMS for k in range(KF)]
                    if has_fcb:
                        ops.append((fcb[:, hc, 0, :], onesb[:, :]))
                        ops.append((fcb[:, hc, 1, :], onesb[:, :]))
                    for i, (l_, r_) in enumerate(ops):
                        mm(ph[:], l_, r_, start=(i == 0),
                           stop=(i == len(ops) - 1))
                    for q in range(NQ):
                        sl = ph[:, q * QB:(q + 1) * QB]
                        nc.vector.tensor_copy(htq[q][:, hc, 0, :], sl)
                        nc.vector.tensor_tensor(
                            out=htq[q][:, hc, 1, :], in0=sl, in1=htq[q][:, hc, 0, :],
                            op=OP.subtract)

            # ---- decode loop: software-pipelined across batch quarters.
            # GHH = ht@Whh gates part (only needs ht -> fills the PE while
            # quarters wait on their sampling chains), ATAIL = et@Wih gates
            # part + LSTM cell + AG(ht), B1 = logits/stats/AG(stats),
            # B2 = combine/embed.
            env = locals()
            for q in range(NQ):
                _gates_hh(nc, env, 0, q)
                _phase_atail(nc, env, 0, q)
            for t in range(steps):
                gt = gp.tile([128, NQ, VS], F32, tag="gum", name=f"gt{t}")
                nc.gpsimd.dma_start(gt[:], gum_d[t])
                env["gt"] = gt
                for q in range(NQ):
                    _phase_b1(nc, env, t, q)
                if t + 1 < steps:
                    for q in range(NQ):
                        _gates_hh(nc, env, t + 1, q)
                for q in range(NQ):
                    _phase_b2(nc, env, t, q)
                    if t + 1 < steps:
                        _phase_atail(nc, env, t + 1, q)

            # ---- write outputs ----
            for q in range(NQ):
                nc.sync.dma_start(msg_d[q * QB:(q + 1) * QB, :], tokbuf[:, q, :])
                nc.sync.dma_start(lp_d[q * QB:(q + 1) * QB, :], lpbuf[:, q, :])
                nc.sync.dma_start(ent_d[q * QB:(q + 1) * QB, :], entbuf[:, q, :])

    if SPLIT_WAITS:
        _split_excess_waits(nc)
    return nc


def _gates_hh(nc, env, t, q):
    """Open the gates psum and accumulate the ht@Whh^T part (24 matmuls).
    Depends only on htq (state t-1) - schedulable while the sampling chain
    of step t-1 is still in flight."""
    mm = nc.tensor.matmul
    pgp = env["pgp"]; whh = env["whh"]; htq = env["htq"]
    sfx = f"_{t}_{q}"

    pg = pgp.tile([128, 512], F32, tag="pg", name="pg" + sfx)
    env[f"pg{q}"] = pg
    first = True
    for k in range(KH):
        for a in (0, 1):
            lhsT = htq[q][:, k, a, :]
            for b2 in ((0, 1) if a == 0 else (0,)):
                mm(pg[:], lhsT, whh[b2][:, k, :], start=first, stop=False)
                first = False


def _phase_atail(nc, env, t, q):
    """et@Wih^T gates part (+bias) -> LSTM cell -> transpose/split -> AG."""
    mm = nc.tensor.matmul
    act = nc.scalar.activation
    dve = nc.vector
    gps = nc.gpsimd

    wk = env["wk"]; sm = env["sm"]; dr = env["dr"]
    ptp = env["ptp"]
    wih = env["wih"]
    gbias = env["gbias"]; onesb = env["onesb"]; idf32 = env["idf32"]
    htq = env["htq"]; etq = env["etq"]; ctq = env["ctq"]
    has_gbias = env["has_gbias"]
    pg = env[f"pg{q}"]
    sfx = f"_{t}_{q}"

    ops = []
    for k in range(KE):
        for a in (0, 1):
            lhsT = etq[q][:, k, a, :]
            for b2 in ((0, 1) if a == 0 else (0,)):
                ops.append((lhsT, wih[b2][:, k, :]))
    if has_gbias:
        ops.append((onesb[:, 0:128], gbias[:, 0, :]))
        ops.append((onesb[:, 0:128], gbias[:, 1, :]))
    for i, (l_, r_) in enumerate(ops):
        mm(pg[:], l_, r_, start=False, stop=(i == len(ops) - 1))

    # LSTM cell (b-major, this core's 128 h-dims)
    sig = wk.tile([128, 512], F32, tag="sig", name="sig" + sfx)
    act(sig[:, 0:256], pg[:, 0:256], AF.Sigmoid)          # i, f
    act(sig[:, 256:384], pg[:, 256:384], AF.Tanh)         # g
    act(sig[:, 384:512], pg[:, 384:512], AF.Sigmoid)      # o
    t1 = sm.tile([128, QB], F32, tag="t1", name="t1" + sfx)
    t2 = sm.tile([128, QB], F32, tag="t2", name="t2" + sfx)
    dve.tensor_tensor(out=t1[:], in0=sig[:, 128:256], in1=ctq[q][:], op=OP.mult)
    dve.tensor_tensor(out=t2[:], in0=sig[:, 0:128], in1=sig[:, 256:384], op=OP.mult)
    dve.tensor_tensor(out=ctq[q][:], in0=t1[:], in1=t2[:], op=OP.add)
    tct = sm.tile([128, QB], F32, tag="tct", name="tct" + sfx)
    act(tct[:], ctq[q][:], AF.Tanh)
    htn = sm.tile([128, QB], F32, tag="htn", name="htn" + sfx)
    dve.tensor_tensor(out=htn[:], in0=sig[:, 384:512], in1=tct[:], op=OP.mult)

    # transpose to h-major, split hi/lo, AllGather
    ptr = ptp.tile([128, 128], F32, tag="tr", name="ptr" + sfx)
    mm(ptr[:], htn[:], idf32[:], is_transpose=True, start=True, stop=True)
    htm = sm.tile([128, 2, QB], BF16, tag="htm", name="htm" + sfx)
    dve.tensor_copy(htm[:, 0, :], ptr[:])
    dve.tensor_tensor(out=htm[:, 1, :], in0=ptr[:], in1=htm[:, 0, :], op=OP.subtract)
    agi = dr.tile([128, 2 * QB], BF16, tag="agi", name="agi" + sfx)
    ago = dr.tile([NCORES * 128, 2 * QB], BF16, tag="ago", name="ago" + sfx)
    nc.scalar.dma_start(agi[:], htm[:])
    gps.collective_compute(
        "AllGather", OP.bypass, ins=[agi[:].opt()], outs=[ago[:].opt()],
        replica_groups=RG)
    nc.sync.dma_start(
        htq[q][:], ago[:].rearrange("(c p) (h b) -> p c h b", c=NCORES, h=2))


def _phase_b1(nc, env, t, q):
    """Logits matmul -> softmax partials + local argmax -> AllGather stats."""
    mm = nc.tensor.matmul
    act = nc.scalar.activation
    dve = nc.vector
    gps = nc.gpsimd

    wk = env["wk"]; sm = env["sm"]; dr = env["dr"]
    plp = env["plp"]
    wout = env["wout"]; bout = env["bout"]; onesb = env["onesb"]
    iota = env["iota"]
    htq = env["htq"]; gt = env["gt"]
    has_bout = env["has_bout"]
    sfx = f"_{t}_{q}"

    pls = [plp.tile([128, 512], F32, tag="pl", name=f"pl{v}" + sfx)
           for v in (0, 1)]
    vops = {0: [], 1: []}
    for k in range(KH):
        for a in (0, 1):
            lhsT = htq[q][:, k, a, :]
            for b2 in ((0, 1) if a == 0 else (0,)):
                for v in (0, 1):
                    vops[v].append((lhsT, wout[b2][:, k, v * 512:(v + 1) * 512]))
    if has_bout:
        for v in (0, 1):
            vops[v].append((onesb[:, 0:128], bout[:, 0, v * 512:(v + 1) * 512]))
            vops[v].append((onesb[:, 0:128], bout[:, 1, v * 512:(v + 1) * 512]))
    n0 = len(vops[0])
    for i in range(n0):
        for v in (0, 1):
            l_, r_ = vops[v][i]
            mm(pls[v][:], l_, r_, start=(i == 0), stop=(i == n0 - 1))

    # drain psum quickly to SBUF (frees the pl banks for the next quarter)
    xab = wk.tile([128, VS], F32, tag="xab", name="xab" + sfx)
    act(xab[:, 0:512], pls[0][:], AF.Copy)
    act(xab[:, 512:1024], pls[1][:], AF.Copy)

    # softmax partials + gumbel argmax candidate
    pk = sm.tile([128, 6], F32, tag="pk", name="pk" + sfx)
    m8 = sm.tile([128, 8], F32, tag="m8", name="m8" + sfx)
    dve.max(m8[:], xab[:])
    dve.tensor_copy(pk[:, 0:1], m8[:, 0:1])                        # m
    negm = sm.tile([128, 1], F32, tag="negm", name="negm" + sfx)
    dve.tensor_scalar_mul(negm[:], m8[:, 0:1], -1.0)
    ete = wk.tile([128, VS], BF16, tag="ete", name="ete" + sfx)
    act(ete[:], xab[:], AF.Exp, bias=negm[:], accum_out=pk[:, 1:2])  # A
    bx = wk.tile([128, VS], F32, tag="bigscr", name="bx" + sfx)
    dve.tensor_tensor(out=bx[:], in0=ete[:], in1=xab[:], op=OP.mult)
    dve.reduce_sum(pk[:, 2:3], bx[:], axis=mybir.AxisListType.X)   # B = sum(e*x)
    sc = wk.tile([128, VS], F32, tag="bigscr", name="sc" + sfx)
    dve.tensor_tensor(out=sc[:], in0=xab[:], in1=gt[:, q, :], op=OP.add)
    s8 = sm.tile([128, 8], F32, tag="s8", name="s8" + sfx)
    i8 = sm.tile([128, 8], mybir.dt.uint32, tag="i8", name="i8" + sfx)
    dve.max(s8[:], sc[:])
    dve.max_index(i8[:], s8[:], sc[:])
    dve.tensor_copy(pk[:, 3:4], s8[:, 0:1])                        # s1
    dve.tensor_copy(pk[:, 4:5], i8[:, 0:1])                        # local idx (f32)
    # x_tok: one-op gather of x' at idx
    gsc = wk.tile([128, VS], F32, tag="bigscr", name="gsc" + sfx)
    dve.scalar_tensor_tensor(
        out=gsc[:], in0=iota[:], scalar=pk[:, 4:5], in1=xab[:],
        op0=OP.is_equal, op1=OP.mult, accum_out=pk[:, 5:6])

    # exchange stats
    sti = dr.tile([128, 6], F32, tag="sti", name="sti" + sfx)
    sto = dr.tile([NCORES * 128, 6], F32, tag="sto", name="sto" + sfx)
    nc.scalar.dma_start(sti[:], pk[:])
    gps.collective_compute(
        "AllGather", OP.bypass, ins=[sti[:].opt()], outs=[sto[:].opt()],
        replica_groups=RG)
    cmb = sm.tile([128, NCORES, 6], F32, tag="cmb", name="cmb" + sfx)
    env[f"cmb{q}"] = cmb
    nc.sync.dma_start(cmb[:], sto[:].rearrange("(c p) x -> p c x", c=NCORES))


def _phase_b2(nc, env, t, q):
    """Combine global stats -> outputs -> embed sampled token."""
    mm = nc.tensor.matmul
    act = nc.scalar.activation
    dve = nc.vector
    gps = nc.gpsimd

    wk = env["wk"]; sm = env["sm"]
    ptp = env["ptp"]
    crev = env["crev"]; idbf = env["idbf"]; emb_d = env["emb_d"]
    etq = env["etq"]
    tokbuf = env["tokbuf"]; lpbuf = env["lpbuf"]; entbuf = env["entbuf"]
    cmb = env[f"cmb{q}"]
    sfx = f"_{t}_{q}"

    # ---------------- combine: global softmax stats + winner ---------------
    mg = sm.tile([128, 1], F32, tag="mg", name="mg" + sfx)
    dve.reduce_max(mg[:], cmb[:, :, 0], axis=mybir.AxisListType.X)
    ngm = sm.tile([128, 1], F32, tag="ngm", name="ngm" + sfx)
    dve.tensor_scalar_mul(ngm[:], mg[:], -1.0)
    wx = sm.tile([128, NCORES], F32, tag="wx", name="wx" + sfx)
    act(wx[:], cmb[:, :, 0], AF.Exp, bias=ngm[:])
    wa = sm.tile([128, NCORES], F32, tag="wa", name="wa" + sfx)
    Ag = sm.tile([128, 1], F32, tag="Ag", name="Ag" + sfx)
    dve.tensor_tensor(out=wa[:], in0=wx[:], in1=cmb[:, :, 1], op=OP.mult)
    dve.reduce_sum(Ag[:], wa[:], axis=mybir.AxisListType.X)
    wb = sm.tile([128, NCORES], F32, tag="wb", name="wb" + sfx)
    Bg = sm.tile([128, 1], F32, tag="Bg", name="Bg" + sfx)
    dve.tensor_tensor(out=wb[:], in0=wx[:], in1=cmb[:, :, 2], op=OP.mult)
    dve.reduce_sum(Bg[:], wb[:], axis=mybir.AxisListType.X)
    sg = sm.tile([128, 1], F32, tag="sg", name="sg" + sfx)
    dve.reduce_max(sg[:], cmb[:, :, 3], axis=mybir.AxisListType.X)
    mk8 = sm.tile([128, NCORES], F32, tag="mk8", name="mk8" + sfx)
    dve.tensor_scalar(mk8[:], cmb[:, :, 3], sg[:], None, op0=OP.is_equal)
    rv = sm.tile([128, NCORES], F32, tag="rv", name="rv" + sfx)
    dve.tensor_tensor(out=rv[:], in0=crev[:], in1=cmb[:, :, 4], op=OP.subtract)
    rvm = sm.tile([128, NCORES], F32, tag="rvm", name="rvm" + sfx)
    trev = sm.tile([128, 1], F32, tag="trev", name="trev" + sfx)
    dve.tensor_tensor(out=rvm[:], in0=mk8[:], in1=rv[:], op=OP.mult)
    dve.reduce_max(trev[:], rvm[:], axis=mybir.AxisListType.X)
    tokf = sm.tile([128, 1], F32, tag="tokf", name="tokf" + sfx)
    dve.tensor_scalar(tokf[:], trev[:], -1.0, 8192.0, op0=OP.mult, op1=OP.add)
    xm = sm.tile([128, NCORES], F32, tag="xm", name="xm" + sfx)
    xg = sm.tile([128, 1], F32, tag="xg", name="xg" + sfx)
    dve.tensor_tensor(out=xm[:], in0=mk8[:], in1=cmb[:, :, 5], op=OP.mult)
    dve.reduce_sum(xg[:], xm[:], axis=mybir.AxisListType.X)
    lnA = sm.tile([128, 1], F32, tag="lnA", name="lnA" + sfx)
    act(lnA[:], Ag[:], AF.Ln)
    lp1 = sm.tile([128, 1], F32, tag="lp1", name="lp1" + sfx)
    dve.tensor_tensor(out=lp1[:], in0=xg[:], in1=mg[:], op=OP.subtract)
    dve.tensor_tensor(out=lpbuf[:, q, t:t + 1], in0=lp1[:], in1=lnA[:],
                      op=OP.subtract)
    rA = sm.tile([128, 1], F32, tag="rA", name="rA" + sfx)
    dve.reciprocal(rA[:], Ag[:])
    bh = sm.tile([128, 1], F32, tag="bh", name="bh" + sfx)
    dve.tensor_tensor(out=bh[:], in0=Bg[:], in1=rA[:], op=OP.mult)
    e1 = sm.tile([128, 1], F32, tag="e1", name="e1" + sfx)
    dve.tensor_tensor(out=e1[:], in0=mg[:], in1=lnA[:], op=OP.add)
    dve.tensor_tensor(out=entbuf[:, q, t:t + 1], in0=e1[:], in1=bh[:],
                      op=OP.subtract)
    dve.tensor_copy(tokbuf[:, q, t:t + 1], tokf[:])

    # ---------------- embed sampled token ---------------------------------
    gix = sm.tile([128, 1], I32, tag="gix", name="gix" + sfx)
    dve.tensor_copy(gix[:], tokf[:])
    etb = wk.tile([128, 2 * E], BF16, tag="etb", name="etb" + sfx)
    gps.indirect_dma_start(
        out=etb[:], out_offset=None, in_=emb_d[:],
        in_offset=IndirectOffsetOnAxis(ap=gix[:, 0:1], axis=0))
    pet = ptp.tile([128, 2 * E], BF16, tag="tr", name="pet" + sfx)
    for j in range(4):
        mm(pet[:, j * 128:(j + 1) * 128], etb[:, j * 128:(j + 1) * 128], idbf[:],
           is_transpose=True, start=(j == 0), stop=(j == 3))
    dve.tensor_copy(
        etq[q][:].rearrange("p k h b -> p h k b"),
        pet[:].rearrange("p (h k b) -> p h k b", h=2, k=2))


# ---------------------------------------------------------------------------
# Host-side input preparation
def _gumbel_host(steps):
    import jax
    import jax.numpy as jnp
    cpu = jax.devices("cpu")[0]
    with jax.default_device(cpu):
        keys = jax.random.split(jax.random.key(42), steps)
        out = np.empty((steps, B, V), np.float32)
        for t in range(steps):
            out[t] = np.asarray(jax.random.gumbel(keys[t], (B, V), jnp.float32))
    return out


def make_in_maps(inputs, steps):
    fc_w = np.asarray(inputs["fc_w"], np.float32)
    fc_b = np.asarray(inputs["fc_b"], np.float32)
    w_ih = np.asarray(inputs["w_ih"], np.float32)
    w_hh = np.asarray(inputs["w_hh"], np.float32)
    b_ih = np.asarray(inputs["b_ih"], np.float32)
    b_hh = np.asarray(inputs["b_hh"], np.float32)
    w_out = np.asarray(inputs["w_out"], np.float32)
    b_out = np.asarray(inputs["b_out"], np.float32)
    emb = np.asarray(inputs["emb"], np.float32)
    sos = np.asarray(inputs["sos"], np.float32)
    fv = np.asarray(inputs["feature_vector"], np.float32)

    G = _gumbel_host(steps)  # [steps, B, V]

    # shared (core-independent) tensors
    ehi, elo = split_pair(emb)
    emb_pair = np.concatenate([ehi, elo], axis=1)  # [V, 2E] bf16
    shi, slo = split_pair(sos)
    et0 = np.zeros((128, KE, 2, QB), bf16)
    for k in range(KE):
        et0[:, k, 0, :] = shi[k * 128:(k + 1) * 128, None]
        et0[:, k, 1, :] = slo[k * 128:(k + 1) * 128, None]
    iota_v = np.broadcast_to(np.arange(VS, dtype=np.float32), (128, VS)).copy()
    crev_a = np.broadcast_to(
        (8192.0 - 1024.0 * np.arange(NCORES, dtype=np.float32)), (128, NCORES)
    ).copy()
    ones_a = np.ones((1, 512), bf16)
    id_bf = np.eye(128, dtype=bf16)
    id_f32 = np.eye(128, dtype=np.float32)

    # fc: [576,1024] -> lhsT [f, h], pad f to 640
    fcwT = np.zeros((KF * 128, H), np.float32)
    fcwT[:F_IN] = fc_w.T
    fhi, flo = split_pair(fcwT)
    fcw_hi = fhi.reshape(KF, 128, KH, 128).transpose(1, 0, 2, 3).copy()
    fcw_lo = flo.reshape(KF, 128, KH, 128).transpose(1, 0, 2, 3).copy()
    bhi, blo = split_pair(fc_b)
    fcb = np.stack([bhi.reshape(KH, 128), blo.reshape(KH, 128)], axis=1)[None]
    fvT = np.zeros((KF * 128, B), np.float32)
    fvT[:F_IN] = fv.T
    vhi, vlo = split_pair(fvT)
    fv_hi = vhi.reshape(KF, 128, B).transpose(1, 0, 2).copy()
    fv_lo = vlo.reshape(KF, 128, B).transpose(1, 0, 2).copy()

    gb = b_ih + b_hh

    in_maps = []
    for c in range(NCORES):
        # gate weights for this core's 128 h-dims: [K, (q,d)] layouts
        selh = w_hh.reshape(4, H, H)[:, c * 128:(c + 1) * 128, :]   # [4,128,H]
        whhT = selh.transpose(2, 0, 1).reshape(KH, 128, 512)        # [kh,p,qd]
        hhi, hlo = split_pair(whhT)
        whh_hi = hhi.transpose(1, 0, 2).copy()
        whh_lo = hlo.transpose(1, 0, 2).copy()
        seli = w_ih.reshape(4, H, E)[:, c * 128:(c + 1) * 128, :]
        wihT = seli.transpose(2, 0, 1).reshape(KE, 128, 512)
        ihi, ilo = split_pair(wihT)
        wih_hi = ihi.transpose(1, 0, 2).copy()
        wih_lo = ilo.transpose(1, 0, 2).copy()
        gsel = gb.reshape(4, H)[:, c * 128:(c + 1) * 128].reshape(512)
        ghi, glo = split_pair(gsel)
        gbias_a = np.stack([ghi, glo])[None]                        # [1,2,512]
        # w_out shard: [H, VS] -> [kh, p, v] -> [p, kh, v]
        woT = w_out[c * VS:(c + 1) * VS, :].T                       # [H, VS]
        ohi, olo = split_pair(woT)
        wout_hi_a = ohi.reshape(KH, 128, VS).transpose(1, 0, 2).copy()
        wout_lo_a = olo.reshape(KH, 128, VS).transpose(1, 0, 2).copy()
        bo = b_out[c * VS:(c + 1) * VS]
        bohi, bolo = split_pair(bo)
        bout_a = np.stack([bohi, bolo])[None]                       # [1,2,VS]
        # gumbel shard: [steps, B, VS] -> [steps, p, q, v]
        gsh = G[:, :, c * VS:(c + 1) * VS]
        gsh = gsh.reshape(steps, NQ, 128, VS).transpose(0, 2, 1, 3).copy()

        in_maps.append({
            "whh_hi": whh_hi, "whh_lo": whh_lo,
            "wih_hi": wih_hi, "wih_lo": wih_lo,
            "gbias": gbias_a,
            "wout_hi": wout_hi_a, "wout_lo": wout_lo_a,
            "bout": bout_a,
            "fcw_hi": fcw_hi, "fcw_lo": fcw_lo, "fcb": fcb,
            "fv_hi": fv_hi, "fv_lo": fv_lo,
            "embp": emb_pair, "et0": et0,
            "gumb": gsh,
            "iotav": iota_v, "crev": crev_a, "onesb": ones_a,
            "idbf": id_bf, "idf32": id_f32,
        })
    return in_maps


_CACHE = {}


def kernel(**inputs):
    steps = int(np.asarray(inputs["message_length"])) - 1
    has_gbias = bool(np.any(np.asarray(inputs["b_ih"]) != 0)
                     or np.any(np.asarray(inputs["b_hh"]) != 0))
    has_bout = bool(np.any(np.asarray(inputs["b_out"]) != 0))
    has_fcb = bool(np.any(np.asarray(inputs["fc_b"]) != 0))
    key = (steps, has_gbias, has_bout, has_fcb)
    if key not in _CACHE:
        _CACHE[key] = build_program(steps, has_gbias, has_bout, has_fcb)
    nc = _CACHE[key]
    in_maps = make_in_maps(inputs, steps)
    res = run_bass_kernel_spmd(
        nc, in_maps, core_ids=list(range(NCORES)),
        trace=bool(int(os.environ.get("KERNEL_TRACE", "0"))))
    out = res.results[0]
    kernel.last_results = res
    msg = out["msg"].astype(np.int32)
    lp = out["lp"].astype(np.float32)
    ent = out["ent"].astype(np.float32)
    return msg, lp, ent


# revision 44
# speedup vs baseline: 1.0985x; 1.0985x over previous
"""Trainium2 Bass kernel for nn_Agent_40063454937396 (LSTM agent decode+sample).

Strategy (8 NeuronCores, model-parallel):
  - Hidden dim H=1024 sharded 8-way: each core computes gates/cell for its 128
    hidden dims over the whole batch, then AllGathers the new hidden state.
  - Vocab V=8192 sharded 8-way for the output projection: each core computes
    logits for its 1024-vocab shard, computes local softmax partials and a
    local Gumbel argmax candidate, then AllGathers 6 scalars per row to
    reproduce exact global sampling + log-prob + entropy.
  - Batch 512 split into 4 independent quarters of 128 rows; the four
    per-quarter chains are pipelined so PE matmuls of one quarter hide the
    collective/sampling latency of the others.
  - All matmuls are dual-bf16 (hi/lo split, 3 terms) for fp32-level accuracy
    at 3 cycles/row instead of fp32's 4.
  - Gumbel noise is precomputed on host CPU with the exact JAX PRNG the
    reference uses (jax.random.gumbel(key, (B, V)) per step).
"""

import os
import sys

for _p in ("/opt/trn_rl_repo", "/root/.axon_site/_ro/trn_rl_repo", "/root/.axon_site"):
    if os.path.isdir(_p) and _p not in sys.path:
        sys.path.append(_p)

# Need the CPU backend alongside axon for host-side PRNG reproduction.
_plat = os.environ.get("JAX_PLATFORMS", "")
if _plat and "cpu" not in _plat:
    os.environ["JAX_PLATFORMS"] = _plat + ",cpu"

import numpy as np
import ml_dtypes

import concourse.bass as bass
import concourse.mybir as mybir
import concourse.tile as tile
from concourse.bass import IndirectOffsetOnAxis
from concourse.bass_utils import run_bass_kernel_spmd
from concourse.vector_clock import ScopedClock

# ---------------------------------------------------------------------------
# The agent image lacks antenv.axon_hooks; bass_utils imports it when
# trace=True under axon. Provide a functional stand-in (NTFF profiling via
# ctypes into libaxon_pjrt.so).
def _install_axon_hooks_shim():
    import contextlib
    import ctypes
    import types

    if "antenv.axon_hooks" in sys.modules:
        return
    try:
        import antenv  # noqa: F401
    except ImportError:
        return
    holder = {"hook": None}

    def set_hook(h):
        holder["hook"] = h

    def get_hook():
        return holder["hook"]

    mod = types.ModuleType("antenv.axon_hooks")
    mod.set_axon_ntff_profile_hook = set_hook
    mod.get_axon_ntff_profile_hook = get_hook
    sys.modules["antenv.axon_hooks"] = mod
    sys.modules["antenv"].axon_hooks = mod

    so_path = "/opt/axon/libaxon_pjrt.so"
    if not os.path.exists(so_path):
        return
    try:
        lib = ctypes.CDLL(so_path)
    except OSError:
        return
    if not hasattr(lib, "axon_start_nrt_profile"):
        return
    lib.axon_start_nrt_profile.argtypes = [
        ctypes.POINTER(ctypes.c_int64), ctypes.c_size_t]
    lib.axon_start_nrt_profile.restype = ctypes.c_int64
    lib.axon_stop_nrt_profile.argtypes = [ctypes.c_char_p]
    lib.axon_stop_nrt_profile.restype = ctypes.c_int64

    @contextlib.contextmanager
    def _hook(output_dir, device_ids):
        import jax
        jax.devices()
        if device_ids:
            ids = (ctypes.c_int64 * len(device_ids))(*device_ids)
            rc = lib.axon_start_nrt_profile(ids, len(device_ids))
        else:
            rc = lib.axon_start_nrt_profile(None, 0)
        if rc != 0:
            raise RuntimeError(f"axon_start_nrt_profile rc={rc}")
        try:
            yield
        finally:
            n = lib.axon_stop_nrt_profile(str(output_dir).encode())
            print(f"profile: {n} file(s) written to {output_dir}", file=sys.stderr)

    set_hook(_hook)


_install_axon_hooks_shim()

F32 = mybir.dt.float32
BF16 = mybir.dt.bfloat16
I32 = mybir.dt.int32
AF = mybir.ActivationFunctionType
OP = mybir.AluOpType

NCORES = 8
B, F_IN, H, E, V = 512, 576, 1024, 256, 8192
HS = H // NCORES          # 128 hidden dims per core
VS = V // NCORES          # 1024 vocab per core
NQ = 4                    # batch quarters
QB = B // NQ              # 128 rows per quarter
KH = H // 128             # 8 contraction chunks over H
KE = E // 128             # 2 contraction chunks over E
KF = 5                    # padded 576 -> 640 = 5 chunks over F_IN
TERMS = ((0, 0), (0, 1), (1, 0))  # (lhs hi/lo, rhs hi/lo) dual-bf16 product terms
RG = [list(range(NCORES))]

bf16 = ml_dtypes.bfloat16


# ---------------------------------------------------------------------------
# walrus rejects instructions with >1 sync wait on the kernel-tail drain;
# split the auto-generated drain waits across multiple drain instructions.
def _patched_drain_and_barrier(self, tick_clock, wait_clock):
    nc = self.nc
    drain_inst = nc.sync.drain()
    wait_clock.add_sem_waits(
        drain_inst.ins, ScopedClock({None: tick_clock.global_clock})
    )
    si = drain_inst.ins.sync_info
    waits = list(si.on_wait)
    if len(waits) > 1:
        si.on_wait = waits[:1]
        for w in waits[1:]:
            d2 = nc.sync.drain()
            d2.ins.sync_info = mybir.SyncInfo(on_wait=[w], on_update=[])
    nc.all_engine_barrier()
    assert self.sems is not None
    popped = nc._tile_sem_poison_stack.pop()
    assert popped is self._sem_poison
    sems = list(self.sems.allocated().values())
    # EVENT_SEMAPHORE_RANGE_CLEAR rejects ranges wider than 16
    for i in range(0, len(sems), 8):
        nc.clear_and_free_semaphores(sems[i:i + 8])
    nc.all_engine_barrier()


tile.TileContext._drain_and_barrier = _patched_drain_and_barrier


def _split_excess_waits(nc, max_waits=1):
    """walrus rejects instructions carrying more than one sync wait; move
    excess waits onto standalone EventSemaphore carriers inserted before."""
    for bb in nc.m.functions[0].blocks:
        il = bb.instructions
        out = []
        changed = False
        for ins in il:
            si = ins.sync_info
            waits = list(si.on_wait) if si else []
            if len(waits) > max_waits:
                changed = True
                keep = waits[:max_waits]
                rest = waits[max_waits:]
                for j, w in enumerate(rest):
                    carrier = mybir.InstNoOp(
                        name=f"{ins.name}_xw{j}", ins=[], outs=[],
                        bass_nofuse=True)
                    carrier.engine = ins.engine
                    carrier.sync_info = mybir.SyncInfo(on_wait=[w], on_update=[])
                    out.append(carrier)
                ins.sync_info = mybir.SyncInfo(
                    on_wait=keep, on_update=list(si.on_update))
            out.append(ins)
        if changed:
            bb.instructions = out


# ---------------------------------------------------------------------------
def split_pair(x):
    """fp32 array -> (hi, lo) bf16 arrays with hi+lo ~= x (2^-17 accurate)."""
    x = np.asarray(x, np.float32)
    hi = x.astype(bf16)
    lo = (x - hi.astype(np.float32)).astype(bf16)
    return hi, lo


SPLIT_WAITS = True  # set False when running under CoreSim (it rejects carriers)


def build_program(steps: int, has_gbias: bool = True, has_bout: bool = True,
                  has_fcb: bool = True):
    nc = bass.Bass(target_bir_lowering=False, trn_type="TRN2")

    dt = nc.dram_tensor
    whh_hi = dt("whh_hi", [128, KH, 512], BF16, kind="ExternalInput")
    whh_lo = dt("whh_lo", [128, KH, 512], BF16, kind="ExternalInput")
    wih_hi = dt("wih_hi", [128, KE, 512], BF16, kind="ExternalInput")
    wih_lo = dt("wih_lo", [128, KE, 512], BF16, kind="ExternalInput")
    gbias_d = dt("gbias", [1, 2, 512], BF16, kind="ExternalInput")
    wout_hi = dt("wout_hi", [128, KH, VS], BF16, kind="ExternalInput")
    wout_lo = dt("wout_lo", [128, KH, VS], BF16, kind="ExternalInput")
    bout_d = dt("bout", [1, 2, VS], BF16, kind="ExternalInput")
    fcw_hi = dt("fcw_hi", [128, KF, KH, 128], BF16, kind="ExternalInput")
    fcw_lo = dt("fcw_lo", [128, KF, KH, 128], BF16, kind="ExternalInput")
    fcb_d = dt("fcb", [1, KH, 2, 128], BF16, kind="ExternalInput")
    fv_hi = dt("fv_hi", [128, KF, B], BF16, kind="ExternalInput")
    fv_lo = dt("fv_lo", [128, KF, B], BF16, kind="ExternalInput")
    emb_d = dt("embp", [V, 2 * E], BF16, kind="ExternalInput")
    et0_d = dt("et0", [128, KE, 2, QB], BF16, kind="ExternalInput")
    gum_d = dt("gumb", [steps, 128, NQ, VS], F32, kind="ExternalInput")
    iota_d = dt("iotav", [128, VS], F32, kind="ExternalInput")
    crev_d = dt("crev", [128, NCORES], F32, kind="ExternalInput")
    ones_d = dt("onesb", [1, 512], BF16, kind="ExternalInput")
    idbf_d = dt("idbf", [128, 128], BF16, kind="ExternalInput")
    idf32_d = dt("idf32", [128, 128], F32, kind="ExternalInput")

    msg_d = dt("msg", [B, steps + 1], I32, kind="ExternalOutput")
    lp_d = dt("lp", [B, steps + 1], F32, kind="ExternalOutput")
    ent_d = dt("ent", [B, steps + 1], F32, kind="ExternalOutput")

    mm = nc.tensor.matmul

    with tile.TileContext(nc) as tc:
        with (
            tc.tile_pool(name="wp", bufs=1) as wp,
            tc.tile_pool(name="st", bufs=1) as st,
            tc.tile_pool(name="gp", bufs=2) as gp,
            tc.tile_pool(name="wk", bufs=3) as wk,
            tc.tile_pool(name="sm", bufs=4) as sm,
            tc.tile_pool(name="pgp", bufs=4, space="PSUM") as pgp,
            tc.tile_pool(name="plp", bufs=2, space="PSUM") as plp,
            tc.tile_pool(name="ptp", bufs=2, space="PSUM") as ptp,
            tc.tile_pool(name="dr", bufs=2, space="DRAM") as dr,
        ):
            # ---- persistent weight tiles ----
            whh = [wp.tile([128, KH, 512], BF16, name=f"whh{i}") for i in range(2)]
            wih = [wp.tile([128, KE, 512], BF16, name=f"wih{i}") for i in range(2)]
            wout = [wp.tile([128, KH, VS], BF16, name=f"wout{i}") for i in range(2)]
            gbias = wp.tile([1, 2, 512], BF16, name="gbias")
            bout = wp.tile([1, 2, VS], BF16, name="bout")
            iota = wp.tile([128, VS], F32, name="iota")
            crev = wp.tile([128, NCORES], F32, name="crev")
            onesb = wp.tile([1, 512], BF16, name="onesb")
            idbf = wp.tile([128, 128], BF16, name="idbf")
            idf32 = wp.tile([128, 128], F32, name="idf32")

            for t_, d_ in (
                (whh[0], whh_hi), (whh[1], whh_lo),
                (wih[0], wih_hi), (wih[1], wih_lo),
                (wout[0], wout_hi), (wout[1], wout_lo),
                (gbias, gbias_d), (bout, bout_d), (iota, iota_d),
                (crev, crev_d), (onesb, ones_d), (idbf, idbf_d),
                (idf32, idf32_d),
            ):
                nc.sync.dma_start(t_[:], d_[:])

            # ---- persistent state ----
            htq = [st.tile([128, KH, 2, QB], BF16, name=f"htq{q}") for q in range(NQ)]
            etq = [st.tile([128, KE, 2, QB], BF16, name=f"etq{q}") for q in range(NQ)]
            ctq = [st.tile([128, QB], F32, name=f"ctq{q}") for q in range(NQ)]
            tokbuf = st.tile([128, NQ, steps + 1], I32, name="tokbuf")
            lpbuf = st.tile([128, NQ, steps + 1], F32, name="lpbuf")
            entbuf = st.tile([128, NQ, steps + 1], F32, name="entbuf")

            for q in range(NQ):
                nc.vector.memset(ctq[q][:], 0.0)
                nc.sync.dma_start(etq[q][:], et0_d[:])
            nc.vector.memset(tokbuf[:], 0)
            nc.vector.memset(lpbuf[:], 0.0)
            nc.vector.memset(entbuf[:], 0.0)

            # ---- init: ht0 = fc(feature_vector) (h-major, full batch) ----
            with tc.tile_pool(name="ip", bufs=1) as ip:
                fcb = ip.tile([1, KH, 2, 128], BF16, name="fcb")
                fv = [ip.tile([128, KF, B], BF16, name=f"fv{i}") for i in range(2)]
                for t_, d_ in ((fcb, fcb_d), (fv[0], fv_hi), (fv[1], fv_lo)):
                    nc.sync.dma_start(t_[:], d_[:])

                for hc in range(KH):
                    fcw = [ip.tile([128, KF, 128], BF16, tag=f"fcw{i}", bufs=2,
                                   name=f"fcw{i}_{hc}") for i in range(2)]
                    nc.sync.dma_start(fcw[0][:], fcw_hi[:, :, hc, :])
                    nc.sync.dma_start(fcw[1][:], fcw_lo[:, :, hc, :])
                    ph = pgp.tile([128, B], F32, tag="pg", name=f"ph{hc}")
                    ops = [(fcw[a][:, k, :], fv[b2][:, k, :])
                           for (a, b2) in TERMS for k in range(KF)]
                    if has_fcb:
                        ops.append((fcb[:, hc, 0, :], onesb[:, :]))
                        ops.append((fcb[:, hc, 1, :], onesb[:, :]))
                    for i, (l_, r_) in enumerate(ops):
                        mm(ph[:], l_, r_, start=(i == 0),
                           stop=(i == len(ops) - 1))
                    for q in range(NQ):
                        sl = ph[:, q * QB:(q + 1) * QB]
                        nc.vector.tensor_copy(htq[q][:, hc, 0, :], sl)
                        nc.vector.tensor_tensor(
                            out=htq[q][:, hc, 1, :], in0=sl, in1=htq[q][:, hc, 0, :],
                            op=OP.subtract)

            # ---- decode loop: software-pipelined across batch quarters.
            # GHH = ht@Whh gates part (only needs ht -> fills the PE while
            # quarters wait on their sampling chains), ATAIL = et@Wih gates
            # part + LSTM cell + AG(ht), B1 = logits/stats/AG(stats),
            # B2 = combine/embed.
            env = locals()
            for q in range(NQ):
                _gates_hh(nc, env, 0, q)
                _phase_atail(nc, env, 0, q)
            for t in range(steps):
                gt = gp.tile([128, NQ, VS], F32, tag="gum", name=f"gt{t}")
                nc.gpsimd.dma_start(gt[:], gum_d[t])
                env["gt"] = gt
                # offset-1 software pipeline: quarter q's combine/embed and
                # next-step gates are emitted right after quarter q+1's
                # logits, so each quarter's sampling tail hides under the
                # next quarter's PE work.
                _phase_b1(nc, env, t, 0)
                _phase_b1(nc, env, t, 1)
                for q in range(NQ):
                    if q + 2 < NQ:
                        _phase_b1(nc, env, t, q + 2)
                    if t + 1 < steps:
                        _gates_hh(nc, env, t + 1, q)
                    _phase_b2(nc, env, t, q)
                    if t + 1 < steps:
                        _phase_atail(nc, env, t + 1, q)

            # ---- write outputs ----
            for q in range(NQ):
                nc.sync.dma_start(msg_d[q * QB:(q + 1) * QB, :], tokbuf[:, q, :])
                nc.sync.dma_start(lp_d[q * QB:(q + 1) * QB, :], lpbuf[:, q, :])
                nc.sync.dma_start(ent_d[q * QB:(q + 1) * QB, :], entbuf[:, q, :])

    if SPLIT_WAITS:
        _split_excess_waits(nc)
    return nc


def _gates_hh(nc, env, t, q):
    """Open the gates psum and accumulate the ht@Whh^T part (24 matmuls).
    Depends only on htq (state t-1) - schedulable while the sampling chain
    of step t-1 is still in flight."""
    mm = nc.tensor.matmul
    pgp = env["pgp"]; whh = env["whh"]; htq = env["htq"]
    sfx = f"_{t}_{q}"

    pg = pgp.tile([128, 512], F32, tag="pg", name="pg" + sfx)
    env[f"pg{q}"] = pg
    first = True
    for k in range(KH):
        for a in (0, 1):
            lhsT = htq[q][:, k, a, :]
            for b2 in ((0, 1) if a == 0 else (0,)):
                mm(pg[:], lhsT, whh[b2][:, k, :], start=first, stop=False)
                first = False


def _phase_atail(nc, env, t, q):
    """et@Wih^T gates part (+bias) -> LSTM cell -> transpose/split -> AG."""
    mm = nc.tensor.matmul
    act = nc.scalar.activation
    dve = nc.vector
    gps = nc.gpsimd

    wk = env["wk"]; sm = env["sm"]; dr = env["dr"]
    ptp = env["ptp"]
    wih = env["wih"]
    gbias = env["gbias"]; onesb = env["onesb"]; idf32 = env["idf32"]
    htq = env["htq"]; etq = env["etq"]; ctq = env["ctq"]
    has_gbias = env["has_gbias"]
    pg = env[f"pg{q}"]
    sfx = f"_{t}_{q}"

    ops = []
    for k in range(KE):
        for a in (0, 1):
            lhsT = etq[q][:, k, a, :]
            for b2 in ((0, 1) if a == 0 else (0,)):
                ops.append((lhsT, wih[b2][:, k, :]))
    if has_gbias:
        ops.append((onesb[:, 0:128], gbias[:, 0, :]))
        ops.append((onesb[:, 0:128], gbias[:, 1, :]))
    for i, (l_, r_) in enumerate(ops):
        mm(pg[:], l_, r_, start=False, stop=(i == len(ops) - 1))

    # LSTM cell (b-major, this core's 128 h-dims)
    sig = wk.tile([128, 512], F32, tag="sig", name="sig" + sfx)
    act(sig[:, 0:256], pg[:, 0:256], AF.Sigmoid)          # i, f
    act(sig[:, 256:384], pg[:, 256:384], AF.Tanh)         # g
    act(sig[:, 384:512], pg[:, 384:512], AF.Sigmoid)      # o
    t1 = sm.tile([128, QB], F32, tag="t1", name="t1" + sfx)
    t2 = sm.tile([128, QB], F32, tag="t2", name="t2" + sfx)
    dve.tensor_tensor(out=t1[:], in0=sig[:, 128:256], in1=ctq[q][:], op=OP.mult)
    dve.tensor_tensor(out=t2[:], in0=sig[:, 0:128], in1=sig[:, 256:384], op=OP.mult)
    dve.tensor_tensor(out=ctq[q][:], in0=t1[:], in1=t2[:], op=OP.add)
    tct = sm.tile([128, QB], F32, tag="tct", name="tct" + sfx)
    act(tct[:], ctq[q][:], AF.Tanh)
    htn = sm.tile([128, QB], F32, tag="htn", name="htn" + sfx)
    dve.tensor_tensor(out=htn[:], in0=sig[:, 384:512], in1=tct[:], op=OP.mult)

    # transpose to h-major, split hi/lo, AllGather
    ptr = ptp.tile([128, 128], F32, tag="tr", name="ptr" + sfx)
    mm(ptr[:], htn[:], idf32[:], is_transpose=True, start=True, stop=True)
    htm = sm.tile([128, 2, QB], BF16, tag="htm", name="htm" + sfx)
    dve.tensor_copy(htm[:, 0, :], ptr[:])
    dve.tensor_tensor(out=htm[:, 1, :], in0=ptr[:], in1=htm[:, 0, :], op=OP.subtract)
    agi = dr.tile([128, 2 * QB], BF16, tag="agi", name="agi" + sfx)
    ago = dr.tile([NCORES * 128, 2 * QB], BF16, tag="ago", name="ago" + sfx)
    nc.scalar.dma_start(agi[:], htm[:])
    gps.collective_compute(
        "AllGather", OP.bypass, ins=[agi[:].opt()], outs=[ago[:].opt()],
        replica_groups=RG)
    nc.sync.dma_start(
        htq[q][:], ago[:].rearrange("(c p) (h b) -> p c h b", c=NCORES, h=2))


def _phase_b1(nc, env, t, q):
    """Logits matmul -> softmax partials + local argmax -> AllGather stats."""
    mm = nc.tensor.matmul
    act = nc.scalar.activation
    dve = nc.vector
    gps = nc.gpsimd

    wk = env["wk"]; sm = env["sm"]; dr = env["dr"]
    plp = env["plp"]
    wout = env["wout"]; bout = env["bout"]; onesb = env["onesb"]
    iota = env["iota"]
    htq = env["htq"]; gt = env["gt"]
    has_bout = env["has_bout"]
    sfx = f"_{t}_{q}"

    pls = [plp.tile([128, 512], F32, tag="pl", name=f"pl{v}" + sfx)
           for v in (0, 1)]
    vops = {0: [], 1: []}
    for k in range(KH):
        for a in (0, 1):
            lhsT = htq[q][:, k, a, :]
            for b2 in ((0, 1) if a == 0 else (0,)):
                for v in (0, 1):
                    vops[v].append((lhsT, wout[b2][:, k, v * 512:(v + 1) * 512]))
    if has_bout:
        for v in (0, 1):
            vops[v].append((onesb[:, 0:128], bout[:, 0, v * 512:(v + 1) * 512]))
            vops[v].append((onesb[:, 0:128], bout[:, 1, v * 512:(v + 1) * 512]))
    n0 = len(vops[0])
    for i in range(n0):
        for v in (0, 1):
            l_, r_ = vops[v][i]
            mm(pls[v][:], l_, r_, start=(i == 0), stop=(i == n0 - 1))

    # drain psum quickly to SBUF (frees the pl banks for the next quarter)
    xab = wk.tile([128, VS], F32, tag="xab", name="xab" + sfx)
    act(xab[:, 0:512], pls[0][:], AF.Copy)
    act(xab[:, 512:1024], pls[1][:], AF.Copy)

    # softmax partials + gumbel argmax candidate
    pk = sm.tile([128, 6], F32, tag="pk", name="pk" + sfx)
    m8 = sm.tile([128, 8], F32, tag="m8", name="m8" + sfx)
    dve.max(m8[:], xab[:])
    dve.tensor_copy(pk[:, 0:1], m8[:, 0:1])                        # m
    negm = sm.tile([128, 1], F32, tag="negm", name="negm" + sfx)
    dve.tensor_scalar_mul(negm[:], m8[:, 0:1], -1.0)
    ete = wk.tile([128, VS], BF16, tag="ete", name="ete" + sfx)
    act(ete[:], xab[:], AF.Exp, bias=negm[:], accum_out=pk[:, 1:2])  # A
    bx = wk.tile([128, VS], F32, tag="bigscr", name="bx" + sfx)
    dve.tensor_tensor(out=bx[:], in0=ete[:], in1=xab[:], op=OP.mult)
    dve.reduce_sum(pk[:, 2:3], bx[:], axis=mybir.AxisListType.X)   # B = sum(e*x)
    sc = wk.tile([128, VS], F32, tag="bigscr", name="sc" + sfx)
    dve.tensor_tensor(out=sc[:], in0=xab[:], in1=gt[:, q, :], op=OP.add)
    s8 = sm.tile([128, 8], F32, tag="s8", name="s8" + sfx)
    i8 = sm.tile([128, 8], mybir.dt.uint32, tag="i8", name="i8" + sfx)
    dve.max(s8[:], sc[:])
    dve.max_index(i8[:], s8[:], sc[:])
    dve.tensor_copy(pk[:, 3:4], s8[:, 0:1])                        # s1
    dve.tensor_copy(pk[:, 4:5], i8[:, 0:1])                        # local idx (f32)
    # x_tok: one-op gather of x' at idx
    gsc = wk.tile([128, VS], F32, tag="bigscr", name="gsc" + sfx)
    dve.scalar_tensor_tensor(
        out=gsc[:], in0=iota[:], scalar=pk[:, 4:5], in1=xab[:],
        op0=OP.is_equal, op1=OP.mult, accum_out=pk[:, 5:6])

    # exchange stats
    sti = dr.tile([128, 6], F32, tag="sti", name="sti" + sfx)
    sto = dr.tile([NCORES * 128, 6], F32, tag="sto", name="sto" + sfx)
    nc.scalar.dma_start(sti[:], pk[:])
    gps.collective_compute(
        "AllGather", OP.bypass, ins=[sti[:].opt()], outs=[sto[:].opt()],
        replica_groups=RG)
    cmb = sm.tile([128, NCORES, 6], F32, tag="cmb", name="cmb" + sfx)
    env[f"cmb{q}"] = cmb
    nc.sync.dma_start(cmb[:], sto[:].rearrange("(c p) x -> p c x", c=NCORES))


def _phase_b2(nc, env, t, q):
    """Combine global stats -> outputs -> embed sampled token."""
    mm = nc.tensor.matmul
    act = nc.scalar.activation
    dve = nc.vector
    gps = nc.gpsimd

    wk = env["wk"]; sm = env["sm"]
    ptp = env["ptp"]
    crev = env["crev"]; idbf = env["idbf"]; emb_d = env["emb_d"]
    etq = env["etq"]
    tokbuf = env["tokbuf"]; lpbuf = env["lpbuf"]; entbuf = env["entbuf"]
    cmb = env[f"cmb{q}"]
    sfx = f"_{t}_{q}"

    # ---------------- combine: global softmax stats + winner ---------------
    mg = sm.tile([128, 1], F32, tag="mg", name="mg" + sfx)
    dve.reduce_max(mg[:], cmb[:, :, 0], axis=mybir.AxisListType.X)
    ngm = sm.tile([128, 1], F32, tag="ngm", name="ngm" + sfx)
    dve.tensor_scalar_mul(ngm[:], mg[:], -1.0)
    wx = sm.tile([128, NCORES], F32, tag="wx", name="wx" + sfx)
    act(wx[:], cmb[:, :, 0], AF.Exp, bias=ngm[:])
    wa = sm.tile([128, NCORES], F32, tag="wa", name="wa" + sfx)
    Ag = sm.tile([128, 1], F32, tag="Ag", name="Ag" + sfx)
    dve.tensor_tensor(out=wa[:], in0=wx[:], in1=cmb[:, :, 1], op=OP.mult)
    dve.reduce_sum(Ag[:], wa[:], axis=mybir.AxisListType.X)
    wb = sm.tile([128, NCORES], F32, tag="wb", name="wb" + sfx)
    Bg = sm.tile([128, 1], F32, tag="Bg", name="Bg" + sfx)
    dve.tensor_tensor(out=wb[:], in0=wx[:], in1=cmb[:, :, 2], op=OP.mult)
    dve.reduce_sum(Bg[:], wb[:], axis=mybir.AxisListType.X)
    sg = sm.tile([128, 1], F32, tag="sg", name="sg" + sfx)
    dve.reduce_max(sg[:], cmb[:, :, 3], axis=mybir.AxisListType.X)
    mk8 = sm.tile([128, NCORES], F32, tag="mk8", name="mk8" + sfx)
    dve.tensor_scalar(mk8[:], cmb[:, :, 3], sg[:], None, op0=OP.is_equal)
    rv = sm.tile([128, NCORES], F32, tag="rv", name="rv" + sfx)
    dve.tensor_tensor(out=rv[:], in0=crev[:], in1=cmb[:, :, 4], op=OP.subtract)
    rvm = sm.tile([128, NCORES], F32, tag="rvm", name="rvm" + sfx)
    trev = sm.tile([128, 1], F32, tag="trev", name="trev" + sfx)
    dve.tensor_tensor(out=rvm[:], in0=mk8[:], in1=rv[:], op=OP.mult)
    dve.reduce_max(trev[:], rvm[:], axis=mybir.AxisListType.X)
    tokf = sm.tile([128, 1], F32, tag="tokf", name="tokf" + sfx)
    dve.tensor_scalar(tokf[:], trev[:], -1.0, 8192.0, op0=OP.mult, op1=OP.add)
    xm = sm.tile([128, NCORES], F32, tag="xm", name="xm" + sfx)
    xg = sm.tile([128, 1], F32, tag="xg", name="xg" + sfx)
    dve.tensor_tensor(out=xm[:], in0=mk8[:], in1=cmb[:, :, 5], op=OP.mult)
    dve.reduce_sum(xg[:], xm[:], axis=mybir.AxisListType.X)
    lnA = sm.tile([128, 1], F32, tag="lnA", name="lnA" + sfx)
    act(lnA[:], Ag[:], AF.Ln)
    lp1 = sm.tile([128, 1], F32, tag="lp1", name="lp1" + sfx)
    dve.tensor_tensor(out=lp1[:], in0=xg[:], in1=mg[:], op=OP.subtract)
    dve.tensor_tensor(out=lpbuf[:, q, t:t + 1], in0=lp1[:], in1=lnA[:],
                      op=OP.subtract)
    rA = sm.tile([128, 1], F32, tag="rA", name="rA" + sfx)
    dve.reciprocal(rA[:], Ag[:])
    bh = sm.tile([128, 1], F32, tag="bh", name="bh" + sfx)
    dve.tensor_tensor(out=bh[:], in0=Bg[:], in1=rA[:], op=OP.mult)
    e1 = sm.tile([128, 1], F32, tag="e1", name="e1" + sfx)
    dve.tensor_tensor(out=e1[:], in0=mg[:], in1=lnA[:], op=OP.add)
    dve.tensor_tensor(out=entbuf[:, q, t:t + 1], in0=e1[:], in1=bh[:],
                      op=OP.subtract)
    dve.tensor_copy(tokbuf[:, q, t:t + 1], tokf[:])

    # ---------------- embed sampled token ---------------------------------
    gix = sm.tile([128, 1], I32, tag="gix", name="gix" + sfx)
    dve.tensor_copy(gix[:], tokf[:])
    etb = wk.tile([128, 2 * E], BF16, tag="etb", name="etb" + sfx)
    gps.indirect_dma_start(
        out=etb[:], out_offset=None, in_=emb_d[:],
        in_offset=IndirectOffsetOnAxis(ap=gix[:, 0:1], axis=0))
    pet = ptp.tile([128, 2 * E], BF16, tag="tr", name="pet" + sfx)
    for j in range(4):
        mm(pet[:, j * 128:(j + 1) * 128], etb[:, j * 128:(j + 1) * 128], idbf[:],
           is_transpose=True, start=(j == 0), stop=(j == 3))
    dve.tensor_copy(
        etq[q][:].rearrange("p k h b -> p h k b"),
        pet[:].rearrange("p (h k b) -> p h k b", h=2, k=2))


# ---------------------------------------------------------------------------
# Host-side input preparation
def _gumbel_host(steps):
    import jax
    import jax.numpy as jnp
    cpu = jax.devices("cpu")[0]
    with jax.default_device(cpu):
        keys = jax.random.split(jax.random.key(42), steps)
        out = np.empty((steps, B, V), np.float32)
        for t in range(steps):
            out[t] = np.asarray(jax.random.gumbel(keys[t], (B, V), jnp.float32))
    return out


def make_in_maps(inputs, steps):
    fc_w = np.asarray(inputs["fc_w"], np.float32)
    fc_b = np.asarray(inputs["fc_b"], np.float32)
    w_ih = np.asarray(inputs["w_ih"], np.float32)
    w_hh = np.asarray(inputs["w_hh"], np.float32)
    b_ih = np.asarray(inputs["b_ih"], np.float32)
    b_hh = np.asarray(inputs["b_hh"], np.float32)
    w_out = np.asarray(inputs["w_out"], np.float32)
    b_out = np.asarray(inputs["b_out"], np.float32)
    emb = np.asarray(inputs["emb"], np.float32)
    sos = np.asarray(inputs["sos"], np.float32)
    fv = np.asarray(inputs["feature_vector"], np.float32)

    G = _gumbel_host(steps)  # [steps, B, V]

    # shared (core-independent) tensors
    ehi, elo = split_pair(emb)
    emb_pair = np.concatenate([ehi, elo], axis=1)  # [V, 2E] bf16
    shi, slo = split_pair(sos)
    et0 = np.zeros((128, KE, 2, QB), bf16)
    for k in range(KE):
        et0[:, k, 0, :] = shi[k * 128:(k + 1) * 128, None]
        et0[:, k, 1, :] = slo[k * 128:(k + 1) * 128, None]
    iota_v = np.broadcast_to(np.arange(VS, dtype=np.float32), (128, VS)).copy()
    crev_a = np.broadcast_to(
        (8192.0 - 1024.0 * np.arange(NCORES, dtype=np.float32)), (128, NCORES)
    ).copy()
    ones_a = np.ones((1, 512), bf16)
    id_bf = np.eye(128, dtype=bf16)
    id_f32 = np.eye(128, dtype=np.float32)

    # fc: [576,1024] -> lhsT [f, h], pad f to 640
    fcwT = np.zeros((KF * 128, H), np.float32)
    fcwT[:F_IN] = fc_w.T
    fhi, flo = split_pair(fcwT)
    fcw_hi = fhi.reshape(KF, 128, KH, 128).transpose(1, 0, 2, 3).copy()
    fcw_lo = flo.reshape(KF, 128, KH, 128).transpose(1, 0, 2, 3).copy()
    bhi, blo = split_pair(fc_b)
    fcb = np.stack([bhi.reshape(KH, 128), blo.reshape(KH, 128)], axis=1)[None]
    fvT = np.zeros((KF * 128, B), np.float32)
    fvT[:F_IN] = fv.T
    vhi, vlo = split_pair(fvT)
    fv_hi = vhi.reshape(KF, 128, B).transpose(1, 0, 2).copy()
    fv_lo = vlo.reshape(KF, 128, B).transpose(1, 0, 2).copy()

    gb = b_ih + b_hh

    in_maps = []
    for c in range(NCORES):
        # gate weights for this core's 128 h-dims: [K, (q,d)] layouts
        selh = w_hh.reshape(4, H, H)[:, c * 128:(c + 1) * 128, :]   # [4,128,H]
        whhT = selh.transpose(2, 0, 1).reshape(KH, 128, 512)        # [kh,p,qd]
        hhi, hlo = split_pair(whhT)
        whh_hi = hhi.transpose(1, 0, 2).copy()
        whh_lo = hlo.transpose(1, 0, 2).copy()
        seli = w_ih.reshape(4, H, E)[:, c * 128:(c + 1) * 128, :]
        wihT = seli.transpose(2, 0, 1).reshape(KE, 128, 512)
        ihi, ilo = split_pair(wihT)
        wih_hi = ihi.transpose(1, 0, 2).copy()
        wih_lo = ilo.transpose(1, 0, 2).copy()
        gsel = gb.reshape(4, H)[:, c * 128:(c + 1) * 128].reshape(512)
        ghi, glo = split_pair(gsel)
        gbias_a = np.stack([ghi, glo])[None]                        # [1,2,512]
        # w_out shard: [H, VS] -> [kh, p, v] -> [p, kh, v]
        woT = w_out[c * VS:(c + 1) * VS, :].T                       # [H, VS]
        ohi, olo = split_pair(woT)
        wout_hi_a = ohi.reshape(KH, 128, VS).transpose(1, 0, 2).copy()
        wout_lo_a = olo.reshape(KH, 128, VS).transpose(1, 0, 2).copy()
        bo = b_out[c * VS:(c + 1) * VS]
        bohi, bolo = split_pair(bo)
        bout_a = np.stack([bohi, bolo])[None]                       # [1,2,VS]
        # gumbel shard: [steps, B, VS] -> [steps, p, q, v]
        gsh = G[:, :, c * VS:(c + 1) * VS]
        gsh = gsh.reshape(steps, NQ, 128, VS).transpose(0, 2, 1, 3).copy()

        in_maps.append({
            "whh_hi": whh_hi, "whh_lo": whh_lo,
            "wih_hi": wih_hi, "wih_lo": wih_lo,
            "gbias": gbias_a,
            "wout_hi": wout_hi_a, "wout_lo": wout_lo_a,
            "bout": bout_a,
            "fcw_hi": fcw_hi, "fcw_lo": fcw_lo, "fcb": fcb,
            "fv_hi": fv_hi, "fv_lo": fv_lo,
            "embp": emb_pair, "et0": et0,
            "gumb": gsh,
            "iotav": iota_v, "crev": crev_a, "onesb": ones_a,
            "idbf": id_bf, "idf32": id_f32,
        })
    return in_maps


_CACHE = {}


def kernel(**inputs):
    steps = int(np.asarray(inputs["message_length"])) - 1
    has_gbias = bool(np.any(np.asarray(inputs["b_ih"]) != 0)
                     or np.any(np.asarray(inputs["b_hh"]) != 0))
    has_bout = bool(np.any(np.asarray(inputs["b_out"]) != 0))
    has_fcb = bool(np.any(np.asarray(inputs["fc_b"]) != 0))
    key = (steps, has_gbias, has_bout, has_fcb)
    if key not in _CACHE:
        _CACHE[key] = build_program(steps, has_gbias, has_bout, has_fcb)
    nc = _CACHE[key]
    in_maps = make_in_maps(inputs, steps)
    res = run_bass_kernel_spmd(
        nc, in_maps, core_ids=list(range(NCORES)),
        trace=bool(int(os.environ.get("KERNEL_TRACE", "0"))))
    out = res.results[0]
    kernel.last_results = res
    msg = out["msg"].astype(np.int32)
    lp = out["lp"].astype(np.float32)
    ent = out["ent"].astype(np.float32)
    return msg, lp, ent
